# revision 1
# baseline (speedup 1.0000x reference)
"""Temporal attention kernel for Trainium2, data-parallel over batch on 8 cores.

Reference computation (B=64, T=256, D=128, H=8, E=128):
    Q = x@Wq + bq; K = x@Wk + bk; V = x@Wv + bv          [B,T,H,E]
    scores  = einsum('bthd,bjhd->bhtj', Q, K)            [B,H,T,T]
    summary = (scale*scores) @ Ws + bs                   [B,H,T,1]
    beta    = softmax(summary, axis=t)                   [B,H,T]
    result  = sum_t V[b,t,h,:] * beta[b,h,t]             [B,H,E]
    out     = result.reshape(B,H*E) @ Wo + bo            [B,D]

Algebraic restructure (exact up to fp reassociation):
  * Ws contracts the key axis j immediately, so K enters only through
      Ks[b,:] = (Ws^T x_b) @ Wk + sum(Ws)*bk             [HE]
    and Q enters only through per-head dots with Ks:
      summary[t,h] = x_b[t,:] @ (scale*Wq[:,hE:hE+E] @ Ks[hE:hE+E])
  * softmax over t is shift-invariant => the bq/bs bias terms (constant in t)
    drop out entirely.  Logits are O(0.05), so exp() without max-subtraction
    is exact; normalization is deferred to the tiny [d,(b,h)] aggregate.
  * V enters only through sum_t beta[t,h] x_b[t,:], since sum_t beta = 1:
      result[h,:] = (beta^T x_b)[h,:] @ Wv[:,hE:] + bv[hE:]
  This removes the [B,H,T,T] scores tensor and all three full projections:
  ~13 GFLOP -> ~140 MFLOP, leaving the kernel DMA-bound (~3MB/core).

All matmuls are oriented so N (moving free dim) stays small (8) and outputs
land in the layout the next stage consumes - the only transposes are x->xT
(16) and the two tiny [128,64] flips around softmax normalization.
"""

import contextlib

import numpy as np

import concourse.bacc as bacc
import concourse.bass as bass
import concourse.mybir as mybir
import concourse.tile as tile
from concourse.bass_utils import run_bass_kernel_spmd

N_CORES = 8
B, T, D = 64, 256, 128
H, E = 8, 128
HE = H * E
BL = B // N_CORES          # samples per core
TC = T // 128              # 128-token chunks per sample (2)
NJ = BL * TC               # token chunks per core (16)
SCALE = 1.0 / float(np.sqrt(np.float32(E)))

FP32 = mybir.dt.float32
AF = mybir.ActivationFunctionType

# consts blob column layout: [ident | ws | bot | bkT | bvT]
C_ID, C_WS, C_BOT, C_BKT, C_BVT = 0, 128, 130, 131, 139
C_TOT = 147

_cached = {}


def _build_program():
    nc = bacc.Bacc("TRN2", target_bir_lowering=False, debug=False)

    x_d = nc.dram_tensor("x", [BL, T, D], FP32, kind="ExternalInput").ap()
    cst_d = nc.dram_tensor("cst", [128, C_TOT], FP32, kind="ExternalInput").ap()
    wk_d = nc.dram_tensor("wk", [D, HE], FP32, kind="ExternalInput").ap()
    wqt_d = nc.dram_tensor("wqt", [HE, D], FP32, kind="ExternalInput").ap()
    wv_d = nc.dram_tensor("wv", [D, HE], FP32, kind="ExternalInput").ap()
    wor_d = nc.dram_tensor("wor", [D, HE], FP32, kind="ExternalInput").ap()
    y_d = nc.dram_tensor("y", [BL, D], FP32, kind="ExternalOutput").ap()

    with tile.TileContext(nc) as tc:
        _emit(tc, x_d, cst_d, wk_d, wqt_d, wv_d, wor_d, y_d)
    nc.compile()
    return nc


def _emit(tc, x_d, cst_d, wk_d, wqt_d, wv_d, wor_d, y_d):
    nc = tc.nc
    with contextlib.ExitStack() as ctx:
        cpool = ctx.enter_context(tc.tile_pool(name="consts", bufs=1))
        ppool = ctx.enter_context(tc.tile_pool(name="psums", bufs=1,
                                               space="PSUM"))

        # ---- persistent SBUF tiles ----
        cst = cpool.tile([128, C_TOT], FP32, tag="cst")
        x_sb = cpool.tile([128, NJ, D], FP32, tag="x")      # [t, (b,c), d]
        xt_sb = cpool.tile([128, NJ, 128], FP32, tag="xt")  # [d, (b,c), t]
        wk_sb = cpool.tile([128, HE], FP32, tag="wk")       # [d, he]
        wqt_sb = cpool.tile([128, H, D], FP32, tag="wqt")   # [e, h, d]
        wv_sb = cpool.tile([128, HE], FP32, tag="wv")       # [d, he]
        wor_sb = cpool.tile([128, HE], FP32, tag="wor")     # wo as [k,(h,d)]

        sws_sb = cpool.tile([128, 1], FP32, tag="sws")      # sum(Ws) bcast
        bkw_sb = cpool.tile([128, H], FP32, tag="bkw")      # bkT * sum(Ws)
        xst_sb = cpool.tile([128, BL], FP32, tag="xst")     # [d, b]
        kst_sb = cpool.tile([128, H, BL], FP32, tag="kst")  # [e, h, b]
        wqh_sb = cpool.tile([128, H, BL], FP32, tag="wqh")  # [d, h, b]
        e_sb = cpool.tile([128, TC, BL, H], FP32, tag="esb")  # [t, c, b, h]
        xbtu_sb = cpool.tile([128, BL, H], FP32, tag="xbtu")  # [d, b, h]
        xbtn_sb = cpool.tile([64, 128], FP32, tag="xbtn")   # [(b,h), d]
        xbt_sb = cpool.tile([128, BL, H], FP32, tag="xbt")  # [d, b, h]
        rec_sb = cpool.tile([64, 1], FP32, tag="rec")       # 1/esum (b,h)
        rest_sb = cpool.tile([128, H, BL], FP32, tag="rest")  # [e, h, b]
        outt_sb = cpool.tile([128, BL], FP32, tag="outt")   # [dout, b]
        y_sb = cpool.tile([BL, D], FP32, tag="ysb")

        ones_sb = cpool.tile([128, 128], FP32, tag="ones")
        ident = cst[:, C_ID:C_ID + 128]
        ones128 = ones_sb[:]
        bot = cst[:, C_BOT:C_BOT + 1]
        bkt = cst[:, C_BKT:C_BKT + H]
        bvt = cst[:, C_BVT:C_BVT + H]

        # ---- input DMAs, in dependency-criticality order ----
        nc.sync.dma_start(cst[:], cst_d)
        nc.vector.memset(ones_sb[:], 1.0)
        xr = x_d.rearrange("b (c t) d -> t (b c) d", t=128)
        for s in range(4):
            nc.sync.dma_start(x_sb[:, s * NJ // 4:(s + 1) * NJ // 4, :],
                              xr[:, s * NJ // 4:(s + 1) * NJ // 4, :])
        nc.sync.dma_start(wk_sb[:], wk_d)
        nc.sync.dma_start(wqt_sb[:], wqt_d.rearrange("(h e) d -> e h d", e=128))
        nc.sync.dma_start(wv_sb[:], wv_d)
        nc.sync.dma_start(wor_sb[:], wor_d)

        # ---- sum(Ws) broadcast down all partitions, then bkw = bkT*sws ----
        sws_ps = ppool.tile([128, 1], FP32, tag="mm8", bufs=1)
        for c in range(TC):
            nc.tensor.matmul(sws_ps[:], ones128, cst[:, C_WS + c:C_WS + c + 1],
                             start=(c == 0), stop=(c == TC - 1))
        nc.vector.tensor_copy(sws_sb[:], sws_ps[:])
        nc.vector.tensor_scalar_mul(bkw_sb[:], bkt, sws_sb[:])

        # ---- xsT[d, b] = sum_t Ws[t] x_b[t, d] ----
        xst_ps = ppool.tile([128, BL], FP32, tag="mm8", bufs=1)
        for b in range(BL):
            for c in range(TC):
                nc.tensor.matmul(xst_ps[:, b:b + 1], x_sb[:, b * TC + c, :],
                                 cst[:, C_WS + c:C_WS + c + 1],
                                 start=(c == 0), stop=(c == TC - 1))
        nc.vector.tensor_copy(xst_sb[:], xst_ps[:])

        # ---- KsT[e, h, b] = Wk_h^T xs + sum(Ws)*bk_h (rank-1 accumulate) ----
        kst_ps = ppool.tile([128, H, BL], FP32, tag="hb64", bufs=1)
        for h in range(H):
            nc.tensor.matmul(kst_ps[:, h, :], wk_sb[:, h * E:(h + 1) * E],
                             xst_sb[:], start=True, stop=True)
        nc.vector.tensor_add(kst_sb[:], kst_ps[:],
                             bkw_sb[:, :, None].broadcast_to([128, H, BL]))

        # ---- WqhT[d, h, b] (scale pre-folded into wqt on host) ----
        wqh_ps = ppool.tile([128, H, BL], FP32, tag="hb64", bufs=1)
        for h in range(H):
            nc.tensor.matmul(wqh_ps[:, h, :], wqt_sb[:, h, :], kst_sb[:, h, :],
                             start=True, stop=True)
        nc.vector.tensor_copy(wqh_sb[:], wqh_ps[:])

        # ---- xT: transpose x chunks, 4 per PSUM bank ----
        for p in range(NJ // 4):
            tp = ppool.tile([128, 512], FP32, tag="tpx", bufs=2)
            for q in range(4):
                nc.tensor.transpose(tp[:, q * 128:(q + 1) * 128],
                                    x_sb[:, 4 * p + q, :], ident)
            if p % 2 == 0:
                nc.vector.tensor_copy(xt_sb[:, 4 * p:4 * p + 4, :], tp[:])
            else:
                nc.scalar.copy(xt_sb[:, 4 * p:4 * p + 4, :], tp[:])

        # ---- summary[t, c, b, h] then E = exp(summary) in one shot ----
        summ_ps = ppool.tile([128, TC, BL, H], FP32, tag="summ", bufs=1)
        for b in range(BL):
            for c in range(TC):
                j = b * TC + c
                nc.tensor.matmul(summ_ps[:, c, b, :], xt_sb[:, j, :],
                                 wqh_sb[:, :, b], start=True, stop=True)
        nc.scalar.activation(e_sb[:], summ_ps[:], AF.Exp)

        # ---- esum[(b,h)] via ones-matmul over t, both chunks ----
        esum_ps = ppool.tile([64, 1], FP32, tag="mm8", bufs=1)
        for c in range(TC):
            nc.tensor.matmul(esum_ps[:], e_sb[:, c], ones128[:, :1],
                             start=(c == 0), stop=(c == TC - 1))
        nc.vector.reciprocal(rec_sb[:], esum_ps[:])

        # ---- xbtU[d, b, h] = sum_t x[t,d] E[t,(b,c),h] ----
        xbtu_ps = ppool.tile([128, BL, H], FP32, tag="xbtu", bufs=1)
        for b in range(BL):
            for c in range(TC):
                j = b * TC + c
                nc.tensor.matmul(xbtu_ps[:, b, :], x_sb[:, j, :],
                                 e_sb[:, c, b, :],
                                 start=(c == 0), stop=(c == TC - 1))
        nc.vector.tensor_copy(xbtu_sb[:], xbtu_ps[:])

        # ---- normalize: transpose, scale rows by 1/esum, transpose back ----
        xbtn_ps = ppool.tile([64, 128], FP32, tag="mm8", bufs=1)
        nc.tensor.transpose(xbtn_ps[:], xbtu_sb.rearrange("d b h -> d (b h)"),
                            ident)
        nc.scalar.activation(xbtn_sb[:], xbtn_ps[:], AF.Copy, scale=rec_sb[:])
        xbt_ps = ppool.tile([128, 64], FP32, tag="mm8", bufs=1)
        nc.tensor.transpose(xbt_ps[:], xbtn_sb[:], ident[:64, :64])
        nc.vector.tensor_copy(xbt_sb.rearrange("d b h -> d (b h)"), xbt_ps[:])

        # ---- resultT[e, h, b] = Wv_h^T xbt[:, :, h] + bv_h (rank-1) ----
        rest_ps = ppool.tile([128, H, BL], FP32, tag="hb64", bufs=1)
        for h in range(H):
            nc.tensor.matmul(rest_ps[:, h, :], wv_sb[:, h * E:(h + 1) * E],
                             xbt_sb[:, :, h], start=True, stop=True)
        nc.vector.tensor_add(rest_sb[:], rest_ps[:],
                             bvt[:, :, None].broadcast_to([128, H, BL]))

        # ---- outT[dout, b] = sum_h Wo_h^T restT[:, h, :], + bo ----
        outt_ps = ppool.tile([128, BL], FP32, tag="outt", bufs=1)
        for h in range(H):
            nc.tensor.matmul(outt_ps[:], wor_sb[:, h * E:(h + 1) * E],
                             rest_sb[:, h, :], start=(h == 0), stop=(h == H - 1))
        nc.scalar.activation(outt_sb[:], outt_ps[:], AF.Identity, bias=bot)

        # ---- y[b, dout]: store via transposing DMA access pattern ----
        nc.sync.dma_start(y_d.rearrange("b d -> d b"), outt_sb[:])


def _prep_in_maps(inputs):
    x = np.ascontiguousarray(inputs["x"], dtype=np.float32)
    Wq = np.asarray(inputs["Wq"], dtype=np.float32)
    Wv = np.asarray(inputs["Wv"], dtype=np.float32)
    Wo = np.asarray(inputs["Wo"], dtype=np.float32)
    Ws = np.asarray(inputs["Ws"], dtype=np.float32).reshape(T)

    cst = np.zeros((128, C_TOT), dtype=np.float32)
    cst[:, C_ID:C_ID + 128] = np.eye(128, dtype=np.float32)
    for c in range(TC):
        cst[:, C_WS + c] = Ws[c * 128:(c + 1) * 128]
    cst[:, C_BOT] = np.asarray(inputs["bo"], dtype=np.float32)
    cst[:, C_BKT:C_BKT + H] = (
        np.asarray(inputs["bk"], dtype=np.float32).reshape(H, E).T)
    cst[:, C_BVT:C_BVT + H] = (
        np.asarray(inputs["bv"], dtype=np.float32).reshape(H, E).T)

    wo_r = Wo.reshape(H, E, D).transpose(1, 0, 2).reshape(E, H * D)
    shared = {
        "cst": cst,
        "wk": np.ascontiguousarray(inputs["Wk"], dtype=np.float32),
        "wqt": np.ascontiguousarray((SCALE * Wq).T),
        "wv": np.ascontiguousarray(Wv),
        "wor": np.ascontiguousarray(wo_r),
    }
    return [
        {"x": np.ascontiguousarray(x[c * BL:(c + 1) * BL]), **shared}
        for c in range(N_CORES)
    ]


def kernel(**inputs):
    if "nc" not in _cached:
        _cached["nc"] = _build_program()
    nc = _cached["nc"]
    in_maps = _prep_in_maps(inputs)
    res = run_bass_kernel_spmd(nc, in_maps, list(range(N_CORES)))
    _cached["last_results"] = res
    return np.concatenate([res.results[c]["y"] for c in range(N_CORES)], axis=0)

